# revision 1
# baseline (speedup 1.0000x reference)
import math

import numpy as np

# Problem constants (nn_Attention_83502754169400): hardcoded per contract.
B, S, D, H = 2, 2048, 2048, 16
HD = D // H          # 128
NCORES = 8
HL = H // NCORES     # heads per core = 2
DL = HL * HD         # per-core projected width = 256
EPS = 1e-5
SCALE = 1.0 / math.sqrt(HD)


def _kernel_jax(inputs):
    import jax
    import jax.numpy as jnp

    devs = jax.devices()[:NCORES]
    assert len(devs) == NCORES

    x = inputs["x"].astype(np.float32)
    fc = inputs["freqs_cos"].astype(np.float32)
    fs = inputs["freqs_sin"].astype(np.float32)
    mask = inputs["mask"].astype(np.float32)
    wq, wk, wv, wo = (inputs[k].astype(np.float32) for k in ("wq", "wk", "wv", "wo"))
    qw, qb = inputs["q_ln_w"].astype(np.float32), inputs["q_ln_b"].astype(np.float32)
    kw, kb = inputs["k_ln_w"].astype(np.float32), inputs["k_ln_b"].astype(np.float32)

    # Tensor-parallel over heads: column-shard wq/wk/wv, row-shard wo.
    wq_s = np.stack([wq[:, c * DL:(c + 1) * DL] for c in range(NCORES)])  # [8, D, DL]
    wk_s = np.stack([wk[:, c * DL:(c + 1) * DL] for c in range(NCORES)])
    wv_s = np.stack([wv[:, c * DL:(c + 1) * DL] for c in range(NCORES)])
    wo_s = np.stack([wo[c * DL:(c + 1) * DL, :] for c in range(NCORES)])  # [8, DL, D]

    def _ln(t, w, b):
        mu = jnp.mean(t, axis=-1, keepdims=True)
        var = jnp.mean(jnp.square(t - mu), axis=-1, keepdims=True)
        return (t - mu) * jax.lax.rsqrt(var + EPS) * w + b

    def _rope(t, c, s):
        e, o = t[..., 0::2], t[..., 1::2]
        cc = c[None, :, None, :]
        ss = s[None, :, None, :]
        oe = e * cc - o * ss
        oo = e * ss + o * cc
        return jnp.stack([oe, oo], axis=-1).reshape(t.shape)

    def shard_fn(wq_c, wk_c, wv_c, wo_c, x_c, fc_c, fs_c, m_c, qw_c, qb_c, kw_c, kb_c):
        b, s, _ = x_c.shape
        q = (x_c.reshape(b * s, D) @ wq_c).reshape(b, s, HL, HD)
        k = (x_c.reshape(b * s, D) @ wk_c).reshape(b, s, HL, HD)
        v = (x_c.reshape(b * s, D) @ wv_c).reshape(b, s, HL, HD)
        q = _ln(q, qw_c, qb_c)
        k = _ln(k, kw_c, kb_c)
        q = _rope(q, fc_c, fs_c)
        k = _rope(k, fc_c, fs_c)
        scores = jnp.einsum("bqhd,bkhd->bhqk", q, k) * SCALE
        scores = scores + m_c[None, None, :, :]
        probs = jax.nn.softmax(scores, axis=-1)
        out = jnp.einsum("bhqk,bkhd->bqhd", probs, v).reshape(b, s, HL * HD)
        part = out.reshape(b * s, HL * HD) @ wo_c
        return jax.lax.psum(part.reshape(b, s, D), "i")

    pfn = jax.pmap(
        shard_fn,
        axis_name="i",
        in_axes=(0, 0, 0, 0, None, None, None, None, None, None, None, None),
        devices=devs,
    )
    res = pfn(wq_s, wk_s, wv_s, wo_s, x, fc, fs, mask, qw, qb, kw, kb)
    return np.asarray(res[0], dtype=np.float32)


def _kernel_numpy(inputs):
    x = inputs["x"].astype(np.float32)
    fc, fs = inputs["freqs_cos"], inputs["freqs_sin"]
    mask = inputs["mask"]
    wq, wk, wv, wo = inputs["wq"], inputs["wk"], inputs["wv"], inputs["wo"]
    qw, qb = inputs["q_ln_w"], inputs["q_ln_b"]
    kw, kb = inputs["k_ln_w"], inputs["k_ln_b"]

    def ln(t, w, b):
        mu = t.mean(-1, keepdims=True)
        var = ((t - mu) ** 2).mean(-1, keepdims=True)
        return (t - mu) / np.sqrt(var + EPS) * w + b

    def rope(t):
        e, o = t[..., 0::2], t[..., 1::2]
        c = fc[None, :, None, :]
        s = fs[None, :, None, :]
        out = np.empty_like(t)
        out[..., 0::2] = e * c - o * s
        out[..., 1::2] = e * s + o * c
        return out

    b, s, _ = x.shape
    q = (x @ wq).reshape(b, s, H, HD)
    k = (x @ wk).reshape(b, s, H, HD)
    v = (x @ wv).reshape(b, s, H, HD)
    q = rope(ln(q, qw, qb))
    k = rope(ln(k, kw, kb))
    out = np.empty((b, s, H, HD), dtype=np.float32)
    for bi in range(b):
        for h in range(H):
            sc = (q[bi, :, h, :] @ k[bi, :, h, :].T) * SCALE + mask
            sc -= sc.max(-1, keepdims=True)
            p = np.exp(sc)
            p /= p.sum(-1, keepdims=True)
            out[bi, :, h, :] = p @ v[bi, :, h, :]
    return (out.reshape(b, s, D) @ wo).astype(np.float32)


def kernel(**inputs) -> np.ndarray:
    try:
        return _kernel_jax(inputs)
    except Exception:
        return _kernel_numpy(inputs)

